# revision 22
# baseline (speedup 1.0000x reference)
"""Trainium2 Bass kernel for nn_Disc_edge_15573551415682 (GNN message passing).

Sharding: data-parallel over batch B=8 -> 8 NeuronCores (1 graph/core).

Device math (per graph, all edge tensors in "pair-tile" layout):
  pair q in [0,128) covers node rows (q, q+128).
  pair-tile = [128 partitions, 256 cols]:
    partitions 0:64   = features of row q      (feature-major)
    partitions 64:128 = features of row q+128
    cols = j (neighbor index)

  Per layer l, per 512-col block g (pairs 2g, 2g+1), PSUM [128,512]:
    MM1: lhsT = BD_l   [128,128] block-diag(We_e ; We_e), rhs = e-tiles
    MM2: lhsT = Wxj2_l [64,128]  (Wxj | Wxj),            rhs = xT tiled x2
    MM3: lhsT = BIG2   [2,128],                          rhs = (A-1) rows
         -> adds (A[i,j]-1)*32768 => relu masks the edge (layers 0,2 only;
            layer 1 garbage in masked cols never crosses columns).
  Eviction (per pair, even->ACT odd->DVE):
    relu(psum + bias_col) -> bf16 e-tile, fused accum_out = row-sums
    (bias_col = Axi[:,i] + be : the sender-node term, constant along j).

Layer 0 input: edge_attr is pre-arranged on the host into the feature-major
pair-tile layout; the device does one contiguous gpsimd cast-DMA (fp32->bf16)
per chunk. x1 (node update) computed on device; mean head MLP on host.
"""

import sys
from contextlib import ExitStack

import numpy as np

sys.path.insert(0, "/opt/trn_rl_repo")

import ml_dtypes  # noqa: E402

import concourse.bacc as bacc  # noqa: E402
import concourse.bass as bass  # noqa: E402
import concourse.tile as tile  # noqa: E402
from concourse import mybir  # noqa: E402
from concourse.bass_utils import run_bass_kernel_spmd  # noqa: E402

BF16 = ml_dtypes.bfloat16
F32 = np.float32

B, N, FN, FE = 8, 256, 64, 64
NPAIR = 128          # pairs (q, q+128)
NBLK = 64            # 512-col blocks (2 pairs each)
QC = 16              # pairs per load chunk (1 MB fp32 per chunk read)
NCHUNK = NPAIR // QC
BIGV = 32768.0

_DT = mybir.dt
_nc_cache = None


def _relu(a):
    return np.maximum(a, 0.0)


def _build_program():
    nc = bacc.Bacc(
        "TRN2", target_bir_lowering=False, debug=False, num_devices=8
    )

    def din(name, shape, dt):
        return nc.dram_tensor(name, shape, dt, kind="ExternalInput").ap()

    def dout(name, shape, dt):
        return nc.dram_tensor(name, shape, dt, kind="ExternalOutput").ap()

    e0d = din("e0", [128, 128 * 256], _DT.float32)
    am1d = din("am1", [2, NPAIR * 256], _DT.bfloat16)
    x0t2d = din("x0t2", [64, 512], _DT.bfloat16)
    bias0d = din("bias0", [128, 128], _DT.float32)
    dinvPd = din("dinvP", [128, 128], _DT.float32)
    bd0d = din("bd0", [128, 128], _DT.bfloat16)
    bd1d = din("bd1", [128, 128], _DT.bfloat16)
    bd2d = din("bd2", [128, 128], _DT.bfloat16)
    w23_0d = din("w23_0", [66, 128], _DT.bfloat16)
    w23r1d = din("w23rep_1", [68, 8192], _DT.bfloat16)
    w23r2d = din("w23rep_2", [68, 8192], _DT.bfloat16)
    ind2d = din("ind2", [2, QC * 256], _DT.bfloat16)
    identd = din("ident", [128, 128], _DT.float32)
    wxibe1d = din("wxibe1", [65, 64], _DT.bfloat16)
    wxibe2d = din("wxibe2", [65, 64], _DT.bfloat16)
    wn0d = din("wn0", [128, 64], _DT.bfloat16)
    bn0cd = din("bn0c", [64, 1], _DT.float32)

    voutd = dout("vcols", [128, 64], _DT.float32)
    x1outd = dout("x1dbg", [64, 256], _DT.bfloat16)
    aggoutd = dout("aggdbg", [128, 128], _DT.float32)


    with tile.TileContext(nc) as tc, ExitStack() as ctx:
        cst = ctx.enter_context(tc.tile_pool(name="cst", bufs=1))
        fmp = ctx.enter_context(tc.tile_pool(name="fm", bufs=3))
        psp = ctx.enter_context(tc.tile_pool(name="ps", bufs=6, space="PSUM"))
        psps = ctx.enter_context(tc.tile_pool(name="pss", bufs=2, space="PSUM"))
        e2p = ctx.enter_context(tc.tile_pool(name="e2s", bufs=4))
        e3p = ctx.enter_context(tc.tile_pool(name="e3s", bufs=4))
        e1pool = ctx.enter_context(tc.tile_pool(name="e1", bufs=1))
        smallp = ctx.enter_context(tc.tile_pool(name="small", bufs=1))

        # ---- constants / weights into SBUF ----
        # first edge chunk starts immediately (SWDGE path, parallel to the
        # HWDGE const loads below) so the PE has work ASAP
        fm0 = fmp.tile([128, QC * 256], _DT.bfloat16, tag="fm", name="fm0")
        nc.gpsimd.dma_start(fm0[:], e0d[:, 0 : QC * 256])

        def cload(ap_dram, shape, dt, tag):
            t = cst.tile(shape, dt, tag=tag, name=tag)
            nc.sync.dma_start(t[:], ap_dram)
            return t

        x0t2 = cload(x0t2d, [64, 512], _DT.bfloat16, "x0t2")
        bias0 = cload(bias0d, [128, 128], _DT.float32, "bias0")
        dinvP = cload(dinvPd, [128, 128], _DT.float32, "dinvP")
        bd = [
            cload(d, [128, 128], _DT.bfloat16, f"bd{i}")
            for i, d in enumerate([bd0d, bd1d, bd2d])
        ]
        w23_0 = cload(w23_0d, [66, 128], _DT.bfloat16, "w23_0")
        w23r1 = cload(w23r1d, [68, 8192], _DT.bfloat16, "w23r1")
        w23r2 = cload(w23r2d, [68, 8192], _DT.bfloat16, "w23r2")
        ident = cload(identd, [128, 128], _DT.float32, "ident")
        wxibe1 = cload(wxibe1d, [65, 64], _DT.bfloat16, "wxibe1")
        wxibe2 = cload(wxibe2d, [65, 64], _DT.bfloat16, "wxibe2")
        wn0 = cload(wn0d, [128, 64], _DT.bfloat16, "wn0")
        bn0c = cload(bn0cd, [64, 1], _DT.float32, "bn0c")

        zeros = cst.tile([128, 256], _DT.bfloat16, tag="zeros")
        nc.vector.memset(zeros[:], 0.0)

        e1 = e1pool.tile([128, NPAIR * 256], _DT.bfloat16, tag="e1")
        aggP = smallp.tile([128, 128], _DT.float32, tag="aggP")
        vcols = smallp.tile([128, 64], _DT.float32, tag="vcols")
        biasL = [
            smallp.tile([128, 128], _DT.float32, tag=f"biasL{l}", name=f"biasL{l}") for l in (1, 2)
        ]
        x1t2 = smallp.tile([64, 512], _DT.bfloat16, tag="x1t2")
        m2r = [
            smallp.tile([68, QC * 256], _DT.bfloat16, tag=f"m2r{s}",
                        name=f"m2r{s}")
            for s in (0, 1)
        ]
        nc.sync.dma_start(m2r[0][66:68, :], ind2d)
        nc.sync.dma_start(m2r[1][66:68, :], ind2d)
        x1o = smallp.tile([65, 256], _DT.bfloat16, tag="x1o")
        nc.vector.memset(x1o[64:65, :], 1.0)

        AF = mybir.ActivationFunctionType
        ALU = mybir.AluOpType

        def seed_xpart(slot, xt2):
            nc.vector.tensor_copy(slot[0:64, 0:512], xt2[:])
            nc.vector.tensor_copy(slot[0:64, 512:1024], slot[0:64, 0:512])
            nc.vector.tensor_copy(slot[0:64, 1024:2048], slot[0:64, 0:1024])
            nc.vector.tensor_copy(slot[0:64, 2048:4096], slot[0:64, 0:2048])

        def evict(psum, cols_out, dest, qpair, bias, agg):
            """psum [128,512] -> dest[:, cols_out:+512] bf16 with relu+bias.
            Per-pair bias columns; even half on ACT, odd half on DVE.
            agg: optional [128,128] accum target (cols qpair, qpair+1)."""
            acc0 = agg[:, qpair : qpair + 1] if agg is not None else None
            acc1 = agg[:, qpair + 1 : qpair + 2] if agg is not None else None
            nc.scalar.activation(
                dest[:, cols_out : cols_out + 256],
                psum[:, 0:256],
                AF.Relu,
                bias=bias[:, qpair : qpair + 1],
                accum_out=acc0,
            )
            nc.vector.scalar_tensor_tensor(
                dest[:, cols_out + 256 : cols_out + 512],
                psum[:, 256:512],
                bias[:, qpair + 1 : qpair + 2],
                zeros[:],
                op0=ALU.add,
                op1=ALU.max,
                accum_out=acc1,
            )

        # ================= PASS A: layer 0 =================
        seed_xpart(m2r[0], x0t2)
        seed_xpart(m2r[1], x0t2)
        for c in range(NCHUNK):
            if c == 0:
                fm = fm0
            else:
                fm = fmp.tile([128, QC * 256], _DT.bfloat16, tag="fm")
                nc.gpsimd.dma_start(
                    fm[:], e0d[:, c * QC * 256 : (c + 1) * QC * 256]
                )
            slot = m2r[c % 2]
            nc.sync.dma_start(
                slot[64:66, :], am1d[:, c * QC * 256 : (c + 1) * QC * 256]
            )

            for gg in range(QC // 2):  # 512-col blocks in this chunk
                g = c * (QC // 2) + gg
                q0 = 2 * g
                ps = psp.tile([128, 512], _DT.float32, tag="ps")
                nc.tensor.matmul(
                    ps[:], bd[0][:], fm[:, gg * 512 : (gg + 1) * 512],
                    start=True, stop=False,
                )
                nc.tensor.matmul(
                    ps[:], w23_0[:], slot[0:66, gg * 512 : (gg + 1) * 512],
                    start=False, stop=True,
                )
                evict(ps, g * 512, e1, q0, bias0, aggP)

        # ================= x1 / per-layer aux =================
        aggs = smallp.tile([128, 128], _DT.bfloat16, tag="aggs")
        nc.vector.tensor_mul(aggs[:], aggP[:], dinvP[:])
        xnrhs = smallp.tile([128, 256], _DT.bfloat16, tag="xnrhs")
        nc.sync.dma_start(xnrhs[0:64, 0:256], x0t2d[:, 0:256])
        # partition shift 0:64 -> 64:128 must go through DMA
        nc.sync.dma_start(xnrhs[64:128, 0:128], aggs[0:64, :])
        nc.vector.tensor_copy(xnrhs[64:128, 128:256], aggs[64:128, :])

        psx = psps.tile([64, 256], _DT.float32, tag="pss", bufs=1)
        nc.tensor.matmul(psx[:], wn0[:], xnrhs[:], start=True, stop=True)
        nc.scalar.activation(
            x1t2[:, 0:256], psx[:], AF.Relu, bias=bn0c[:, 0:1]
        )
        nc.vector.tensor_copy(x1t2[:, 256:512], x1t2[:, 0:256])
        nc.vector.tensor_copy(x1o[0:64, :], x1t2[:, 0:256])
        nc.sync.dma_start(x1outd, x1t2[:, 0:256])
        aggcp = smallp.tile([128, 128], _DT.float32, tag="aggcp")
        nc.vector.tensor_copy(aggcp[:], aggP[:])
        nc.sync.dma_start(aggoutd, aggcp[:])

        for li, wxibe, w23r in ((0, wxibe1, w23r1), (1, wxibe2, w23r2)):
            psa = psps.tile([64, 256], _DT.float32, tag="pss", bufs=1)
            nc.tensor.matmul(psa[:], wxibe[:], x1o[:], start=True, stop=True)
            axl = smallp.tile([64, 256], _DT.float32, tag=f"axl{li}", name=f"axl{li}")
            nc.scalar.activation(axl[:], psa[:], AF.Copy)
            nc.vector.tensor_copy(biasL[li][0:64, :], axl[:, 0:128])
            nc.sync.dma_start(biasL[li][64:128, :], axl[:, 128:256])
            # xi-bias as matmul K-rows: transpose biasL -> [pair, m], cast
            # bf16, then scatter pairs into per-block lhsT columns
            pst = psps.tile([128, 128], _DT.float32, tag="psT", name=f"psT{li}", bufs=1)
            nc.tensor.transpose(pst[:], biasL[li][:], ident[:])
            blt = smallp.tile([128, 128], _DT.bfloat16, tag=f"blt{li}",
                              name=f"blt{li}")
            nc.scalar.activation(blt[:], pst[:], AF.Copy)
            for gb in range(64):
                nc.sync.dma_start(
                    w23r[66:68, gb * 128 : (gb + 1) * 128],
                    blt[2 * gb : 2 * gb + 2, :],
                )

        # ================= PASS B: layers 1+2, skewed pipeline =================
        seed_xpart(m2r[0], x1t2)
        e2tiles = {}
        slots_b = {}

        def evict512(psum, dest, g, acc, parity=0):
            """Single [128,512] bias-free relu eviction; alternate engines."""
            accap = acc[:, g : g + 1] if acc is not None else None
            if (g + parity) % 2 == 0:
                nc.scalar.activation(
                    dest[:], psum[:], AF.Relu, accum_out=accap
                )
            else:
                nc.vector.tensor_scalar(
                    dest[:], psum[:], 0.0, 0.0,
                    op0=ALU.max, op1=ALU.add, accum_out=accap,
                )

        def stage_l1(g):
            if g % 8 == 0:
                slot = m2r[(g // 8) % 2]
                nc.sync.dma_start(
                    slot[64:66, :],
                    am1d[:, g * 512 : g * 512 + QC * 256],
                )
                slots_b[g // 8] = slot
            slot = slots_b[g // 8]
            ps1 = psp.tile([128, 512], _DT.float32, tag="ps", name=f"psB1_{g}")
            nc.tensor.matmul(
                ps1[:], bd[1][:], e1[:, g * 512 : (g + 1) * 512],
                start=True, stop=False,
            )
            nc.tensor.matmul(
                ps1[:], w23r1[:, g * 128 : (g + 1) * 128],
                slot[:, (g % 8) * 512 : (g % 8 + 1) * 512],
                start=False, stop=True,
            )
            e2s = e2p.tile([128, 512], _DT.bfloat16, tag="e2s", name=f"e2s_{g}")
            evict512(ps1, e2s, g, None)
            e2tiles[g] = e2s

        def stage_l2(g):
            slot = slots_b[g // 8]
            e2s = e2tiles.pop(g)
            ps2 = psp.tile([128, 512], _DT.float32, tag="ps", name=f"psB2_{g}")
            nc.tensor.matmul(ps2[:], bd[2][:], e2s[:], start=True, stop=False)
            nc.tensor.matmul(
                ps2[:], w23r2[:, g * 128 : (g + 1) * 128],
                slot[:, (g % 8) * 512 : (g % 8 + 1) * 512],
                start=False, stop=True,
            )
            e3s = e3p.tile([128, 512], _DT.bfloat16, tag="e3s", name=f"e3s_{g}")
            evict512(ps2, e3s, g, vcols, parity=1)

        SKEW = 2
        for g in range(NBLK + SKEW):
            if g == 1:
                seed_xpart(m2r[1], x1t2)
            if g < NBLK:
                stage_l1(g)
            if g >= SKEW:
                stage_l2(g - SKEW)

        vcp = smallp.tile([128, 64], _DT.float32, tag="vcp")
        nc.vector.tensor_copy(vcp[:], vcols[:])
        nc.sync.dma_start(voutd, vcp[:])

    nc.compile()
    return nc


def _get_nc():
    global _nc_cache
    if _nc_cache is None:
        _nc_cache = _build_program()
    return _nc_cache


def _prep_core_inputs(b, edge_index, x, edge_attr, weights):
    (We0, be0, Wn0, bn0, We1, be1, We2, be2) = weights
    A = edge_index[b].astype(F32)
    x0 = x[b].astype(F32)

    A2 = A.reshape(2, 128, 256)                       # [r, q, j]
    am1 = (A2 - 1.0).reshape(2, NPAIR * 256).astype(BF16)

    x0t = x0.T.astype(F32)                            # [64, 256]
    x0t2 = np.tile(x0t, (1, 2)).astype(BF16)

    Axi0 = (x0 @ We0[0:64]).T + be0[:, None]          # [64, 256]
    bias0 = np.concatenate([Axi0[:, 0:128], Axi0[:, 128:256]], 0).astype(F32)

    deg = np.clip(A.sum(1), 1.0, None)
    dinv = (1.0 / deg).astype(F32)
    dinvP = np.concatenate(
        [np.tile(dinv[None, 0:128], (64, 1)), np.tile(dinv[None, 128:256], (64, 1))], 0
    ).astype(F32)

    def bdiag(We):
        Wee = We[128:192]
        out = np.zeros((128, 128), F32)
        out[0:64, 0:64] = Wee
        out[64:128, 64:128] = Wee
        return out.astype(BF16)

    big2 = np.zeros((2, 128), F32)
    big2[0, 0:64] = BIGV
    big2[1, 64:128] = BIGV

    def w23(We, masked):
        wxj2 = np.tile(We[64:128], (1, 2))
        rows = big2 if masked else np.zeros((2, 128), F32)
        return np.concatenate([wxj2, rows], 0).astype(BF16)

    def w23rep(We, masked):
        base = w23(We, masked).astype(F32)          # [66, 128]
        rep = np.tile(base, (1, 64))                # [66, 8192]
        out = np.zeros((68, 8192), F32)
        out[0:66] = rep
        return out.astype(BF16)

    ind2 = np.zeros((2, QC * 256), F32)
    ind2[0].reshape(8, 512)[:, 0:256] = 1.0
    ind2[1].reshape(8, 512)[:, 256:512] = 1.0

    return {
        # host pre-arrangement into feature-major pair-tiles:
        # e0[r*64+f, q*256+j] = edge_attr[q+128r, j, f]
        "e0": np.ascontiguousarray(
            edge_attr[b].astype(F32)
            .reshape(2, 128, 256, FE)
            .transpose(0, 3, 1, 2)
            .reshape(128, 128 * 256)
        ),
        "am1": am1,
        "x0t2": x0t2,
        "bias0": bias0,
        "dinvP": dinvP,
        "bd0": bdiag(We0),
        "bd1": bdiag(We1),
        "bd2": bdiag(We2),
        "w23_0": w23(We0, True),
        "w23rep_1": w23rep(We1, False),
        "w23rep_2": w23rep(We2, True),
        "ind2": ind2.astype(BF16),
        "ident": np.eye(128, dtype=F32),
        "wxibe1": np.concatenate([We1[0:64], be1[None, :]], 0).astype(BF16),
        "wxibe2": np.concatenate([We2[0:64], be2[None, :]], 0).astype(BF16),
        "wn0": Wn0.astype(BF16),
        "bn0c": bn0[:, None].astype(F32),
    }


def run_traced(edge_index, x, edge_attr,
               We0, be0, Wn0, bn0,
               We1, be1, Wn1, bn1,
               We2, be2, Wn2, bn2,
               W1, b1, W2, b2, W3, b3, **kw):
    """Correctness + profiling run; returns (out, BassKernelResults)."""
    nc = _get_nc()
    weights = tuple(
        np.asarray(w, F32)
        for w in (We0, be0, Wn0, bn0, We1, be1, We2, be2)
    )
    in_maps = [
        _prep_core_inputs(b, np.asarray(edge_index), np.asarray(x),
                          np.asarray(edge_attr), weights)
        for b in range(B)
    ]
    res = run_bass_kernel_spmd(
        nc, in_maps, core_ids=list(range(B)), trace=True
    )
    return res


def kernel(edge_index, x, edge_attr,
           We0, be0, Wn0, bn0,
           We1, be1, Wn1, bn1,
           We2, be2, Wn2, bn2,
           W1, b1, W2, b2, W3, b3, **kw):
    nc = _get_nc()
    weights = tuple(
        np.asarray(w, F32)
        for w in (We0, be0, Wn0, bn0, We1, be1, We2, be2)
    )
    in_maps = [
        _prep_core_inputs(b, np.asarray(edge_index), np.asarray(x),
                          np.asarray(edge_attr), weights)
        for b in range(B)
    ]
    res = run_bass_kernel_spmd(nc, in_maps, core_ids=list(range(B)))
    out = np.zeros((B,), F32)
    for b in range(B):
        vc = res.results[b]["vcols"].astype(F32)
        v128 = vc.sum(1)
        v = (v128[:64] + v128[64:]) / float(N * N)
        h = _relu(v @ np.asarray(W1, F32) + np.asarray(b1, F32))
        h = _relu(h @ np.asarray(W2, F32) + np.asarray(b2, F32))
        out[b] = (h @ np.asarray(W3, F32) + np.asarray(b3, F32))[0]
    return out


# revision 26
# speedup vs baseline: 1.4902x; 1.4902x over previous
"""Trainium2 Bass kernel for nn_Disc_edge_15573551415682 (GNN message passing).

Sharding: data-parallel over batch B=8 -> 8 NeuronCores (1 graph/core).

Device math (per graph, all edge tensors in "pair-tile" layout):
  pair q in [0,128) covers node rows (q, q+128).
  pair-tile = [128 partitions, 256 cols]:
    partitions 0:64   = features of row q      (feature-major)
    partitions 64:128 = features of row q+128
    cols = j (neighbor index)

  Per layer l, per 512-col block g (pairs 2g, 2g+1), PSUM [128,512]:
    MM1: lhsT = BD_l   [128,128] block-diag(We_e ; We_e), rhs = e-tiles
    MM2: lhsT = Wxj2_l [64,128]  (Wxj | Wxj),            rhs = xT tiled x2
    MM3: lhsT = BIG2   [2,128],                          rhs = (A-1) rows
         -> adds (A[i,j]-1)*32768 => relu masks the edge (layers 0,2 only;
            layer 1 garbage in masked cols never crosses columns).
  Eviction (per pair, even->ACT odd->DVE):
    relu(psum + bias_col) -> bf16 e-tile, fused accum_out = row-sums
    (bias_col = Axi[:,i] + be : the sender-node term, constant along j).

Layer 0 input: edge_attr is pre-arranged on the host into the feature-major
pair-tile layout; the device does one contiguous gpsimd cast-DMA (fp32->bf16)
per chunk. x1 (node update) computed on device; mean head MLP on host.
"""

import sys
from contextlib import ExitStack

import numpy as np

sys.path.insert(0, "/opt/trn_rl_repo")

import ml_dtypes  # noqa: E402

import concourse.bacc as bacc  # noqa: E402
import concourse.bass as bass  # noqa: E402
import concourse.tile as tile  # noqa: E402
from concourse import mybir  # noqa: E402
from concourse.bass_utils import run_bass_kernel_spmd  # noqa: E402

BF16 = ml_dtypes.bfloat16
F32 = np.float32

B, N, FN, FE = 8, 256, 64, 64
NPAIR = 128          # pairs (q, q+128)
NBLK = 64            # 512-col blocks (2 pairs each)
QC = 16              # pairs per load chunk (1 MB fp32 per chunk read)
NCHUNK = NPAIR // QC
BIGV = 32768.0

_DT = mybir.dt
_nc_cache = None


def _relu(a):
    return np.maximum(a, 0.0)


def _build_program():
    nc = bacc.Bacc(
        "TRN2", target_bir_lowering=False, debug=False, num_devices=8
    )

    def din(name, shape, dt):
        return nc.dram_tensor(name, shape, dt, kind="ExternalInput").ap()

    def dout(name, shape, dt):
        return nc.dram_tensor(name, shape, dt, kind="ExternalOutput").ap()

    e0d = din("e0", [128, 128 * 256], _DT.float32)
    am1d = din("am1", [2, NPAIR * 256], _DT.bfloat16)
    x0t2d = din("x0t2", [64, 512], _DT.bfloat16)
    bias0d = din("bias0", [128, 128], _DT.float32)
    dinvPd = din("dinvP", [128, 128], _DT.float32)
    bd0d = din("bd0", [128, 128], _DT.bfloat16)
    bd1d = din("bd1", [128, 128], _DT.bfloat16)
    bd2d = din("bd2", [128, 128], _DT.bfloat16)
    w23_0d = din("w23_0", [66, 128], _DT.bfloat16)
    w23r1d = din("w23rep_1", [68, 8192], _DT.bfloat16)
    w23r2d = din("w23rep_2", [68, 8192], _DT.bfloat16)
    ind2d = din("ind2", [2, QC * 256], _DT.bfloat16)
    identd = din("ident", [128, 128], _DT.float32)
    wxibe1d = din("wxibe1", [65, 64], _DT.bfloat16)
    wxibe2d = din("wxibe2", [65, 64], _DT.bfloat16)
    wn0d = din("wn0", [128, 64], _DT.bfloat16)
    bn0cd = din("bn0c", [64, 1], _DT.float32)

    voutd = dout("vcols", [128, 64], _DT.float32)
    x1outd = dout("x1dbg", [64, 256], _DT.bfloat16)
    aggoutd = dout("aggdbg", [128, 128], _DT.float32)


    with tile.TileContext(nc) as tc, ExitStack() as ctx:
        cst = ctx.enter_context(tc.tile_pool(name="cst", bufs=1))
        fmp = ctx.enter_context(tc.tile_pool(name="fm", bufs=3))
        psp = ctx.enter_context(tc.tile_pool(name="ps", bufs=6, space="PSUM"))
        psps = ctx.enter_context(tc.tile_pool(name="pss", bufs=2, space="PSUM"))
        e2p = ctx.enter_context(tc.tile_pool(name="e2s", bufs=4))
        e3p = ctx.enter_context(tc.tile_pool(name="e3s", bufs=4))
        e1pool = ctx.enter_context(tc.tile_pool(name="e1", bufs=1))
        smallp = ctx.enter_context(tc.tile_pool(name="small", bufs=1))

        # ---- constants / weights into SBUF ----
        # first edge chunk starts immediately (SWDGE path, parallel to the
        # HWDGE const loads below) so the PE has work ASAP
        fm0 = fmp.tile([128, QC * 256], _DT.bfloat16, tag="fm", name="fm0")
        nc.gpsimd.dma_start(fm0[:], e0d[:, 0 : QC * 256])

        def cload(ap_dram, shape, dt, tag):
            t = cst.tile(shape, dt, tag=tag, name=tag)
            nc.sync.dma_start(t[:], ap_dram)
            return t

        x0t2 = cload(x0t2d, [64, 512], _DT.bfloat16, "x0t2")
        bias0 = cload(bias0d, [128, 128], _DT.float32, "bias0")
        dinvP = cload(dinvPd, [128, 128], _DT.float32, "dinvP")
        bd = [
            cload(d, [128, 128], _DT.bfloat16, f"bd{i}")
            for i, d in enumerate([bd0d, bd1d, bd2d])
        ]
        w23_0 = cload(w23_0d, [66, 128], _DT.bfloat16, "w23_0")
        w23r1 = cload(w23r1d, [68, 8192], _DT.bfloat16, "w23r1")
        w23r2 = cload(w23r2d, [68, 8192], _DT.bfloat16, "w23r2")
        ident = cload(identd, [128, 128], _DT.float32, "ident")
        wxibe1 = cload(wxibe1d, [65, 64], _DT.bfloat16, "wxibe1")
        wxibe2 = cload(wxibe2d, [65, 64], _DT.bfloat16, "wxibe2")
        wn0 = cload(wn0d, [128, 64], _DT.bfloat16, "wn0")
        bn0c = cload(bn0cd, [64, 1], _DT.float32, "bn0c")

        zeros = cst.tile([128, 256], _DT.bfloat16, tag="zeros")
        nc.vector.memset(zeros[:], 0.0)

        e1 = e1pool.tile([128, NPAIR * 256], _DT.bfloat16, tag="e1")
        aggP = smallp.tile([128, 128], _DT.float32, tag="aggP")
        vcols = smallp.tile([128, 64], _DT.float32, tag="vcols")
        biasL = [
            smallp.tile([128, 128], _DT.float32, tag=f"biasL{l}", name=f"biasL{l}") for l in (1, 2)
        ]
        x1t2 = smallp.tile([64, 512], _DT.bfloat16, tag="x1t2")
        m2r = [
            smallp.tile([68, QC * 256], _DT.bfloat16, tag=f"m2r{s}",
                        name=f"m2r{s}")
            for s in (0, 1)
        ]
        nc.sync.dma_start(m2r[0][66:68, :], ind2d)
        nc.sync.dma_start(m2r[1][66:68, :], ind2d)
        x1o = smallp.tile([65, 256], _DT.bfloat16, tag="x1o")
        nc.vector.memset(x1o[64:65, :], 1.0)

        AF = mybir.ActivationFunctionType
        ALU = mybir.AluOpType

        def seed_xpart(slot, xt2):
            nc.vector.tensor_copy(slot[0:64, 0:512], xt2[:])
            nc.vector.tensor_copy(slot[0:64, 512:1024], slot[0:64, 0:512])
            nc.vector.tensor_copy(slot[0:64, 1024:2048], slot[0:64, 0:1024])
            nc.vector.tensor_copy(slot[0:64, 2048:4096], slot[0:64, 0:2048])

        def evict(psum, cols_out, dest, qpair, bias, agg):
            """psum [128,512] -> dest[:, cols_out:+512] bf16 with relu+bias.
            Per-pair bias columns; even half on ACT, odd half on DVE.
            agg: optional [128,128] accum target (cols qpair, qpair+1)."""
            acc0 = agg[:, qpair : qpair + 1] if agg is not None else None
            acc1 = agg[:, qpair + 1 : qpair + 2] if agg is not None else None
            nc.scalar.activation(
                dest[:, cols_out : cols_out + 256],
                psum[:, 0:256],
                AF.Relu,
                bias=bias[:, qpair : qpair + 1],
                accum_out=acc0,
            )
            nc.vector.scalar_tensor_tensor(
                dest[:, cols_out + 256 : cols_out + 512],
                psum[:, 256:512],
                bias[:, qpair + 1 : qpair + 2],
                zeros[:],
                op0=ALU.add,
                op1=ALU.max,
                accum_out=acc1,
            )

        # ================= PASS A: layer 0 =================
        seed_xpart(m2r[0], x0t2)
        seed_xpart(m2r[1], x0t2)
        for c in range(NCHUNK):
            if c == 0:
                fm = fm0
            else:
                fm = fmp.tile([128, QC * 256], _DT.bfloat16, tag="fm")
                nc.gpsimd.dma_start(
                    fm[:], e0d[:, c * QC * 256 : (c + 1) * QC * 256]
                )
            slot = m2r[c % 2]
            nc.sync.dma_start(
                slot[64:66, :], am1d[:, c * QC * 256 : (c + 1) * QC * 256]
            )

            for gg in range(QC // 2):  # 512-col blocks in this chunk
                g = c * (QC // 2) + gg
                q0 = 2 * g
                ps = psp.tile([128, 512], _DT.float32, tag="ps")
                nc.tensor.matmul(
                    ps[:], bd[0][:], fm[:, gg * 512 : (gg + 1) * 512],
                    start=True, stop=False,
                )
                nc.tensor.matmul(
                    ps[:], w23_0[:], slot[0:66, gg * 512 : (gg + 1) * 512],
                    start=False, stop=True,
                )
                evict(ps, g * 512, e1, q0, bias0, aggP)

        # ================= x1 / per-layer aux =================
        aggs = smallp.tile([128, 128], _DT.bfloat16, tag="aggs")
        nc.vector.tensor_mul(aggs[:], aggP[:], dinvP[:])
        xnrhs = smallp.tile([128, 256], _DT.bfloat16, tag="xnrhs")
        nc.sync.dma_start(xnrhs[0:64, 0:256], x0t2d[:, 0:256])
        # partition shift 0:64 -> 64:128 must go through DMA
        nc.sync.dma_start(xnrhs[64:128, 0:128], aggs[0:64, :])
        nc.vector.tensor_copy(xnrhs[64:128, 128:256], aggs[64:128, :])

        psx = psps.tile([64, 256], _DT.float32, tag="pss", bufs=1)
        nc.tensor.matmul(psx[:], wn0[:], xnrhs[:], start=True, stop=True)
        nc.scalar.activation(
            x1t2[:, 0:256], psx[:], AF.Relu, bias=bn0c[:, 0:1]
        )
        nc.vector.tensor_copy(x1t2[:, 256:512], x1t2[:, 0:256])
        nc.vector.tensor_copy(x1o[0:64, :], x1t2[:, 0:256])
        nc.sync.dma_start(x1outd, x1t2[:, 0:256])
        aggcp = smallp.tile([128, 128], _DT.float32, tag="aggcp")
        nc.vector.tensor_copy(aggcp[:], aggP[:])
        nc.sync.dma_start(aggoutd, aggcp[:])

        for li, wxibe, w23r in ((0, wxibe1, w23r1), (1, wxibe2, w23r2)):
            psa = psps.tile([64, 256], _DT.float32, tag="pss", bufs=1)
            nc.tensor.matmul(psa[:], wxibe[:], x1o[:], start=True, stop=True)
            axl = smallp.tile([64, 256], _DT.float32, tag=f"axl{li}", name=f"axl{li}")
            nc.scalar.activation(axl[:], psa[:], AF.Copy)
            # biasL columns in (r, g)-major order: col r*64+g = pair 2g+r,
            # so the transposed tensor has even pairs in partitions 0:64 and
            # odd pairs in 64:128 -> contiguous scatter DMAs below.
            nc.vector.tensor_copy(
                biasL[li][0:64, :].rearrange("p (r g) -> p r g", r=2),
                axl[:, 0:128].rearrange("p (g r) -> p r g", r=2),
            )
            axb = axl[:, 128:256].rearrange("p (g r) -> p g r", r=2)
            for r in range(2):
                nc.sync.dma_start(
                    biasL[li][64:128, 64 * r : 64 * r + 64],
                    axb[:, :, r],
                )
            # xi-bias as matmul K-rows: transpose biasL -> [pair, m], cast
            # bf16, then scatter pairs into per-block lhsT columns
            pst = psps.tile([128, 128], _DT.float32, tag="psT", name=f"psT{li}", bufs=1)
            nc.tensor.transpose(pst[:], biasL[li][:], ident[:])
            blt = smallp.tile([128, 128], _DT.bfloat16, tag=f"blt{li}",
                              name=f"blt{li}")
            nc.scalar.activation(blt[:], pst[:], AF.Copy)
            for r in range(2):
                nc.sync.dma_start(
                    w23r[66 + r : 67 + r, :],
                    blt[64 * r : 64 * r + 64, :],
                )

        # ================= PASS B: layers 1+2, skewed pipeline =================
        seed_xpart(m2r[0], x1t2)
        e2tiles = {}
        slots_b = {}

        def evict512(psum, dest, g, acc, parity=0):
            """Single [128,512] bias-free relu eviction; alternate engines."""
            accap = acc[:, g : g + 1] if acc is not None else None
            if (g + parity) % 2 == 0:
                nc.scalar.activation(
                    dest[:], psum[:], AF.Relu, accum_out=accap
                )
            else:
                nc.vector.tensor_scalar(
                    dest[:], psum[:], 0.0, 0.0,
                    op0=ALU.max, op1=ALU.add, accum_out=accap,
                )

        def stage_l1(g):
            if g % 8 == 0:
                slot = m2r[(g // 8) % 2]
                nc.sync.dma_start(
                    slot[64:66, :],
                    am1d[:, g * 512 : g * 512 + QC * 256],
                )
                slots_b[g // 8] = slot
            slot = slots_b[g // 8]
            ps1 = psp.tile([128, 512], _DT.float32, tag="ps", name=f"psB1_{g}")
            nc.tensor.matmul(
                ps1[:], bd[1][:], e1[:, g * 512 : (g + 1) * 512],
                start=True, stop=False,
            )
            nc.tensor.matmul(
                ps1[:], w23r1[:, g * 128 : (g + 1) * 128],
                slot[:, (g % 8) * 512 : (g % 8 + 1) * 512],
                start=False, stop=True,
            )
            e2s = e2p.tile([128, 512], _DT.bfloat16, tag="e2s", name=f"e2s_{g}")
            evict512(ps1, e2s, g, None)
            e2tiles[g] = e2s

        def stage_l2(g):
            slot = slots_b[g // 8]
            e2s = e2tiles.pop(g)
            ps2 = psp.tile([128, 512], _DT.float32, tag="ps", name=f"psB2_{g}")
            nc.tensor.matmul(ps2[:], bd[2][:], e2s[:], start=True, stop=False)
            nc.tensor.matmul(
                ps2[:], w23r2[:, g * 128 : (g + 1) * 128],
                slot[:, (g % 8) * 512 : (g % 8 + 1) * 512],
                start=False, stop=True,
            )
            e3s = e3p.tile([128, 512], _DT.bfloat16, tag="e3s", name=f"e3s_{g}")
            evict512(ps2, e3s, g, vcols, parity=1)

        SKEW = 2
        for g in range(NBLK + SKEW):
            if g == 1:
                seed_xpart(m2r[1], x1t2)
            if g < NBLK:
                stage_l1(g)
            if g >= SKEW:
                stage_l2(g - SKEW)

        vcp = smallp.tile([128, 64], _DT.float32, tag="vcp")
        nc.vector.tensor_copy(vcp[:], vcols[:])
        nc.sync.dma_start(voutd, vcp[:])

    nc.compile()
    return nc


def _get_nc():
    global _nc_cache
    if _nc_cache is None:
        _nc_cache = _build_program()
    return _nc_cache


def _prep_core_inputs(b, edge_index, x, edge_attr, weights):
    (We0, be0, Wn0, bn0, We1, be1, We2, be2) = weights
    A = edge_index[b].astype(F32)
    x0 = x[b].astype(F32)

    A2 = A.reshape(2, 128, 256)                       # [r, q, j]
    am1 = (A2 - 1.0).reshape(2, NPAIR * 256).astype(BF16)

    x0t = x0.T.astype(F32)                            # [64, 256]
    x0t2 = np.tile(x0t, (1, 2)).astype(BF16)

    Axi0 = (x0 @ We0[0:64]).T + be0[:, None]          # [64, 256]
    bias0 = np.concatenate([Axi0[:, 0:128], Axi0[:, 128:256]], 0).astype(F32)

    deg = np.clip(A.sum(1), 1.0, None)
    dinv = (1.0 / deg).astype(F32)
    dinvP = np.concatenate(
        [np.tile(dinv[None, 0:128], (64, 1)), np.tile(dinv[None, 128:256], (64, 1))], 0
    ).astype(F32)

    def bdiag(We):
        Wee = We[128:192]
        out = np.zeros((128, 128), F32)
        out[0:64, 0:64] = Wee
        out[64:128, 64:128] = Wee
        return out.astype(BF16)

    big2 = np.zeros((2, 128), F32)
    big2[0, 0:64] = BIGV
    big2[1, 64:128] = BIGV

    def w23(We, masked):
        wxj2 = np.tile(We[64:128], (1, 2))
        rows = big2 if masked else np.zeros((2, 128), F32)
        return np.concatenate([wxj2, rows], 0).astype(BF16)

    def w23rep(We, masked):
        base = w23(We, masked).astype(F32)          # [66, 128]
        rep = np.tile(base, (1, 64))                # [66, 8192]
        out = np.zeros((68, 8192), F32)
        out[0:66] = rep
        return out.astype(BF16)

    ind2 = np.zeros((2, QC * 256), F32)
    ind2[0].reshape(8, 512)[:, 0:256] = 1.0
    ind2[1].reshape(8, 512)[:, 256:512] = 1.0

    return {
        # host pre-arrangement into feature-major pair-tiles:
        # e0[r*64+f, q*256+j] = edge_attr[q+128r, j, f]
        "e0": np.ascontiguousarray(
            edge_attr[b].astype(F32)
            .reshape(2, 128, 256, FE)
            .transpose(0, 3, 1, 2)
            .reshape(128, 128 * 256)
        ),
        "am1": am1,
        "x0t2": x0t2,
        "bias0": bias0,
        "dinvP": dinvP,
        "bd0": bdiag(We0),
        "bd1": bdiag(We1),
        "bd2": bdiag(We2),
        "w23_0": w23(We0, True),
        "w23rep_1": w23rep(We1, False),
        "w23rep_2": w23rep(We2, True),
        "ind2": ind2.astype(BF16),
        "ident": np.eye(128, dtype=F32),
        "wxibe1": np.concatenate([We1[0:64], be1[None, :]], 0).astype(BF16),
        "wxibe2": np.concatenate([We2[0:64], be2[None, :]], 0).astype(BF16),
        "wn0": Wn0.astype(BF16),
        "bn0c": bn0[:, None].astype(F32),
    }


def run_traced(edge_index, x, edge_attr,
               We0, be0, Wn0, bn0,
               We1, be1, Wn1, bn1,
               We2, be2, Wn2, bn2,
               W1, b1, W2, b2, W3, b3, **kw):
    """Correctness + profiling run; returns (out, BassKernelResults)."""
    nc = _get_nc()
    weights = tuple(
        np.asarray(w, F32)
        for w in (We0, be0, Wn0, bn0, We1, be1, We2, be2)
    )
    in_maps = [
        _prep_core_inputs(b, np.asarray(edge_index), np.asarray(x),
                          np.asarray(edge_attr), weights)
        for b in range(B)
    ]
    res = run_bass_kernel_spmd(
        nc, in_maps, core_ids=list(range(B)), trace=True
    )
    return res


def kernel(edge_index, x, edge_attr,
           We0, be0, Wn0, bn0,
           We1, be1, Wn1, bn1,
           We2, be2, Wn2, bn2,
           W1, b1, W2, b2, W3, b3, **kw):
    nc = _get_nc()
    weights = tuple(
        np.asarray(w, F32)
        for w in (We0, be0, Wn0, bn0, We1, be1, We2, be2)
    )
    in_maps = [
        _prep_core_inputs(b, np.asarray(edge_index), np.asarray(x),
                          np.asarray(edge_attr), weights)
        for b in range(B)
    ]
    res = run_bass_kernel_spmd(nc, in_maps, core_ids=list(range(B)))
    out = np.zeros((B,), F32)
    for b in range(B):
        vc = res.results[b]["vcols"].astype(F32)
        v128 = vc.sum(1)
        v = (v128[:64] + v128[64:]) / float(N * N)
        h = _relu(v @ np.asarray(W1, F32) + np.asarray(b1, F32))
        h = _relu(h @ np.asarray(W2, F32) + np.asarray(b2, F32))
        out[b] = (h @ np.asarray(W3, F32) + np.asarray(b3, F32))[0]
    return out


# revision 31
# speedup vs baseline: 1.6121x; 1.0818x over previous
"""Trainium2 Bass kernel for nn_Disc_edge_15573551415682 (GNN message passing).

Sharding: data-parallel over batch B=8 -> 8 NeuronCores (1 graph/core).

Device math (per graph, all edge tensors in "pair-tile" layout):
  pair q in [0,128) covers node rows (q, q+128).
  pair-tile = [128 partitions, 256 cols]:
    partitions 0:64   = features of row q      (feature-major)
    partitions 64:128 = features of row q+128
    cols = j (neighbor index)

  Per layer l, per 512-col block g (pairs 2g, 2g+1), PSUM [128,512]:
    MM1: lhsT = BD_l   [128,128] block-diag(We_e ; We_e), rhs = e-tiles
    MM2: lhsT = Wxj2_l [64,128]  (Wxj | Wxj),            rhs = xT tiled x2
    MM3: lhsT = BIG2   [2,128],                          rhs = (A-1) rows
         -> adds (A[i,j]-1)*32768 => relu masks the edge (layers 0,2 only;
            layer 1 garbage in masked cols never crosses columns).
  Eviction (per pair, even->ACT odd->DVE):
    relu(psum + bias_col) -> bf16 e-tile, fused accum_out = row-sums
    (bias_col = Axi[:,i] + be : the sender-node term, constant along j).

Layer 0 input: edge_attr is pre-arranged on the host into the feature-major
pair-tile layout; the device does one contiguous gpsimd cast-DMA (fp32->bf16)
per chunk. x1 (node update) computed on device; mean head MLP on host.
"""

import sys
from contextlib import ExitStack

import numpy as np

sys.path.insert(0, "/opt/trn_rl_repo")

import ml_dtypes  # noqa: E402

import concourse.bacc as bacc  # noqa: E402
import concourse.bass as bass  # noqa: E402
import concourse.tile as tile  # noqa: E402
from concourse import mybir  # noqa: E402
from concourse.bass_utils import run_bass_kernel_spmd  # noqa: E402

BF16 = ml_dtypes.bfloat16
F32 = np.float32

B, N, FN, FE = 8, 256, 64, 64
NPAIR = 128          # pairs (q, q+128)
NBLK = 64            # 512-col blocks (2 pairs each)
QC = 16              # pairs per load chunk (1 MB fp32 per chunk read)
NCHUNK = NPAIR // QC
BIGV = 32768.0

_DT = mybir.dt
_nc_cache = None


def _relu(a):
    return np.maximum(a, 0.0)


def _build_program():
    nc = bacc.Bacc(
        "TRN2", target_bir_lowering=False, debug=False, num_devices=8
    )

    def din(name, shape, dt):
        return nc.dram_tensor(name, shape, dt, kind="ExternalInput").ap()

    def dout(name, shape, dt):
        return nc.dram_tensor(name, shape, dt, kind="ExternalOutput").ap()

    e0d = din("e0", [128, 128 * 256], _DT.float32)
    am1d = din("am1", [2, NPAIR * 256], _DT.bfloat16)
    x0t2d = din("x0t2", [64, 512], _DT.bfloat16)
    bias0d = din("bias0", [128, 128], _DT.float32)
    dinvPd = din("dinvP", [128, 128], _DT.float32)
    bd0d = din("bd0", [128, 128], _DT.bfloat16)
    bd1d = din("bd1", [128, 128], _DT.bfloat16)
    bd2d = din("bd2", [128, 128], _DT.bfloat16)
    w23_0d = din("w23_0", [66, 128], _DT.bfloat16)
    w23r1d = din("w23rep_1", [68, 8192], _DT.bfloat16)
    w23r2d = din("w23rep_2", [68, 8192], _DT.bfloat16)
    ind2d = din("ind2", [2, QC * 256], _DT.bfloat16)
    wxibe1d = din("wxibe1", [65, 64], _DT.bfloat16)
    wxibe2d = din("wxibe2", [65, 64], _DT.bfloat16)
    wn0xd = din("wn0x", [64, 64], _DT.bfloat16)
    wn0ad = din("wn0a", [64, 64], _DT.bfloat16)
    wn0a2d = din("wn0a2", [128, 64], _DT.bfloat16)
    bn0cd = din("bn0c", [64, 1], _DT.float32)

    voutd = dout("vcols", [128, 64], _DT.float32)


    with tile.TileContext(nc) as tc, ExitStack() as ctx:
        cst = ctx.enter_context(tc.tile_pool(name="cst", bufs=1))
        fmp = ctx.enter_context(tc.tile_pool(name="fm", bufs=3))
        psp = ctx.enter_context(tc.tile_pool(name="ps", bufs=6, space="PSUM"))
        psps = ctx.enter_context(tc.tile_pool(name="pss", bufs=2, space="PSUM"))
        e2p = ctx.enter_context(tc.tile_pool(name="e2s", bufs=4))
        e3p = ctx.enter_context(tc.tile_pool(name="e3s", bufs=4))
        e1pool = ctx.enter_context(tc.tile_pool(name="e1", bufs=1))
        smallp = ctx.enter_context(tc.tile_pool(name="small", bufs=1))

        # ---- constants / weights into SBUF ----
        # first edge chunk starts immediately (SWDGE path, parallel to the
        # HWDGE const loads below) so the PE has work ASAP
        fm0 = fmp.tile([128, QC * 256], _DT.bfloat16, tag="fm", name="fm0")
        nc.gpsimd.dma_start(fm0[:], e0d[:, 0 : QC * 256])

        def cload(ap_dram, shape, dt, tag):
            t = cst.tile(shape, dt, tag=tag, name=tag)
            nc.sync.dma_start(t[:], ap_dram)
            return t

        x0t2 = cload(x0t2d, [64, 512], _DT.bfloat16, "x0t2")
        bias0 = cload(bias0d, [128, 128], _DT.float32, "bias0")
        dinvP = cload(dinvPd, [128, 128], _DT.float32, "dinvP")
        bd = [
            cload(d, [128, 128], _DT.bfloat16, f"bd{i}")
            for i, d in enumerate([bd0d, bd1d, bd2d])
        ]
        w23_0 = cload(w23_0d, [66, 128], _DT.bfloat16, "w23_0")
        w23r1 = cload(w23r1d, [68, 8192], _DT.bfloat16, "w23r1")
        w23r2 = cload(w23r2d, [68, 8192], _DT.bfloat16, "w23r2")
        wxibe1 = cload(wxibe1d, [65, 64], _DT.bfloat16, "wxibe1")
        wxibe2 = cload(wxibe2d, [65, 64], _DT.bfloat16, "wxibe2")
        wn0x = cload(wn0xd, [64, 64], _DT.bfloat16, "wn0x")
        wn0a = cload(wn0ad, [64, 64], _DT.bfloat16, "wn0a")
        wn0a2 = cload(wn0a2d, [128, 64], _DT.bfloat16, "wn0a2")
        bn0c = cload(bn0cd, [64, 1], _DT.float32, "bn0c")

        zeros = cst.tile([128, 256], _DT.bfloat16, tag="zeros")
        nc.vector.memset(zeros[:], 0.0)

        e1 = e1pool.tile([128, NPAIR * 256], _DT.bfloat16, tag="e1")
        aggP = smallp.tile([128, 128], _DT.float32, tag="aggP")
        vcols = smallp.tile([128, 64], _DT.float32, tag="vcols")
        x1t2 = smallp.tile([64, 512], _DT.bfloat16, tag="x1t2")
        m2r = [
            smallp.tile([68, QC * 256], _DT.bfloat16, tag=f"m2r{s}",
                        name=f"m2r{s}")
            for s in (0, 1)
        ]
        nc.sync.dma_start(m2r[0][66:68, :], ind2d)
        nc.sync.dma_start(m2r[1][66:68, :], ind2d)
        x1o = smallp.tile([65, 256], _DT.bfloat16, tag="x1o")
        nc.vector.memset(x1o[64:65, :], 1.0)

        AF = mybir.ActivationFunctionType
        ALU = mybir.AluOpType

        def seed_xpart(slot, xt2):
            nc.vector.tensor_copy(slot[0:64, 0:512], xt2[:])
            nc.vector.tensor_copy(slot[0:64, 512:1024], slot[0:64, 0:512])
            nc.vector.tensor_copy(slot[0:64, 1024:2048], slot[0:64, 0:1024])
            nc.vector.tensor_copy(slot[0:64, 2048:4096], slot[0:64, 0:2048])

        def evict(psum, cols_out, dest, qpair, bias, agg):
            """psum [128,512] -> dest[:, cols_out:+512] bf16 with relu+bias.
            Per-pair bias columns; even half on ACT, odd half on DVE.
            agg: optional [128,128] accum target (cols qpair, qpair+1)."""
            acc0 = agg[:, qpair : qpair + 1] if agg is not None else None
            acc1 = agg[:, qpair + 1 : qpair + 2] if agg is not None else None
            nc.scalar.activation(
                dest[:, cols_out : cols_out + 256],
                psum[:, 0:256],
                AF.Relu,
                bias=bias[:, qpair : qpair + 1],
                accum_out=acc0,
            )
            nc.vector.scalar_tensor_tensor(
                dest[:, cols_out + 256 : cols_out + 512],
                psum[:, 256:512],
                bias[:, qpair + 1 : qpair + 2],
                zeros[:],
                op0=ALU.add,
                op1=ALU.max,
                accum_out=acc1,
            )

        # ================= PASS A: layer 0 =================
        seed_xpart(m2r[0], x0t2)
        seed_xpart(m2r[1], x0t2)
        for c in range(NCHUNK):
            if c == 0:
                fm = fm0
            else:
                fm = fmp.tile([128, QC * 256], _DT.bfloat16, tag="fm")
                nc.gpsimd.dma_start(
                    fm[:], e0d[:, c * QC * 256 : (c + 1) * QC * 256]
                )
            slot = m2r[c % 2]
            nc.sync.dma_start(
                slot[64:66, :], am1d[:, c * QC * 256 : (c + 1) * QC * 256]
            )

            for gg in range(QC // 2):  # 512-col blocks in this chunk
                g = c * (QC // 2) + gg
                q0 = 2 * g
                ps = psp.tile([128, 512], _DT.float32, tag="ps")
                nc.tensor.matmul(
                    ps[:], bd[0][:], fm[:, gg * 512 : (gg + 1) * 512],
                    start=True, stop=False,
                )
                nc.tensor.matmul(
                    ps[:], w23_0[:], slot[0:66, gg * 512 : (gg + 1) * 512],
                    start=False, stop=True,
                )
                evict(ps, g * 512, e1, q0, bias0, aggP)

        # ================= x1 / per-layer aux =================
        aggs = smallp.tile([128, 128], _DT.bfloat16, tag="aggs")
        nc.vector.tensor_mul(aggs[:], aggP[:], dinvP[:])

        psxa = psps.tile([64, 128], _DT.float32, tag="pss", bufs=1)
        nc.tensor.matmul(
            psxa[:], wn0x[:], x0t2[:, 0:128], start=True, stop=False
        )
        nc.tensor.matmul(
            psxa[:], wn0a[:], aggs[0:64, :], start=False, stop=True
        )
        psxb = psps.tile([64, 128], _DT.float32, tag="psT", bufs=1)
        nc.tensor.matmul(
            psxb[:], wn0x[:], x0t2[:, 128:256], start=True, stop=False
        )
        nc.tensor.matmul(
            psxb[:], wn0a2[64:128, :], aggs[64:128, :],
            start=False, stop=True,
        )
        nc.scalar.activation(
            x1t2[:, 0:128], psxa[:], AF.Relu, bias=bn0c[:, 0:1]
        )
        nc.scalar.activation(
            x1t2[:, 128:256], psxb[:], AF.Relu, bias=bn0c[:, 0:1]
        )
        nc.vector.tensor_copy(x1t2[:, 256:512], x1t2[:, 0:256])
        nc.vector.tensor_copy(x1o[0:64, :], x1t2[:, 0:256])

        # blt[p = r*64+g, f + 64*half] = Axi[f, 2g+r + 128*half] + be:
        # built from (r, g)-major column-gathered x1 (materialized once)
        x1g = smallp.tile([65, 256], _DT.bfloat16, tag="x1g")
        for h in range(2):
            nc.vector.tensor_copy(
                x1g[:, 128 * h : 128 * h + 128].rearrange(
                    "k (r g) -> k r g", r=2
                ),
                x1o[:, 128 * h : 128 * h + 128].rearrange(
                    "k (g r) -> k r g", r=2
                ),
            )
        x1oa = x1g[:, 0:128]
        x1ob = x1g[:, 128:256]
        for li, wxibe, w23r in ((0, wxibe1, w23r1), (1, wxibe2, w23r2)):
            psbl_a = psps.tile([128, 64], _DT.float32, tag="psT",
                               name=f"psbla{li}", bufs=1)
            nc.tensor.matmul(psbl_a[:], x1oa, wxibe[:], start=True, stop=True)
            psbl_b = psps.tile([128, 64], _DT.float32, tag="pss",
                               name=f"psblb{li}", bufs=1)
            nc.tensor.matmul(psbl_b[:], x1ob, wxibe[:], start=True, stop=True)
            blt = smallp.tile([128, 128], _DT.bfloat16, tag=f"blt{li}",
                              name=f"blt{li}")
            nc.scalar.activation(blt[:, 0:64], psbl_a[:], AF.Copy)
            nc.scalar.activation(blt[:, 64:128], psbl_b[:], AF.Copy)
            for r in range(2):
                nc.sync.dma_start(
                    w23r[66 + r : 67 + r, :],
                    blt[64 * r : 64 * r + 64, :],
                )

        # ================= PASS B: layers 1+2, skewed pipeline =================
        seed_xpart(m2r[0], x1t2)
        e2tiles = {}
        slots_b = {}

        def evict512(psum, dest, g, acc, parity=0):
            """Single [128,512] bias-free relu eviction; alternate engines."""
            accap = acc[:, g : g + 1] if acc is not None else None
            if (g + parity) % 2 == 0:
                nc.scalar.activation(
                    dest[:], psum[:], AF.Relu, accum_out=accap
                )
            else:
                nc.vector.tensor_scalar(
                    dest[:], psum[:], 0.0, 0.0,
                    op0=ALU.max, op1=ALU.add, accum_out=accap,
                )

        def stage_l1(g):
            if g % 8 == 0:
                slot = m2r[(g // 8) % 2]
                nc.sync.dma_start(
                    slot[64:66, :],
                    am1d[:, g * 512 : g * 512 + QC * 256],
                )
                slots_b[g // 8] = slot
            slot = slots_b[g // 8]
            ps1 = psp.tile([128, 512], _DT.float32, tag="ps", name=f"psB1_{g}")
            nc.tensor.matmul(
                ps1[:], bd[1][:], e1[:, g * 512 : (g + 1) * 512],
                start=True, stop=False,
            )
            nc.tensor.matmul(
                ps1[:], w23r1[:, g * 128 : (g + 1) * 128],
                slot[:, (g % 8) * 512 : (g % 8 + 1) * 512],
                start=False, stop=True,
            )
            e2s = e2p.tile([128, 512], _DT.bfloat16, tag="e2s", name=f"e2s_{g}")
            evict512(ps1, e2s, g, None)
            e2tiles[g] = e2s

        def stage_l2(g):
            slot = slots_b[g // 8]
            e2s = e2tiles.pop(g)
            ps2 = psp.tile([128, 512], _DT.float32, tag="ps", name=f"psB2_{g}")
            nc.tensor.matmul(ps2[:], bd[2][:], e2s[:], start=True, stop=False)
            nc.tensor.matmul(
                ps2[:], w23r2[:, g * 128 : (g + 1) * 128],
                slot[:, (g % 8) * 512 : (g % 8 + 1) * 512],
                start=False, stop=True,
            )
            e3s = e3p.tile([128, 512], _DT.bfloat16, tag="e3s", name=f"e3s_{g}")
            evict512(ps2, e3s, g, vcols, parity=1)

        SKEW = 2
        for g in range(NBLK + SKEW):
            if g == 1:
                seed_xpart(m2r[1], x1t2)
            if g < NBLK:
                stage_l1(g)
            if g >= SKEW:
                stage_l2(g - SKEW)

        vcp = smallp.tile([128, 64], _DT.float32, tag="vcp")
        nc.vector.tensor_copy(vcp[:], vcols[:])
        nc.sync.dma_start(voutd, vcp[:])

    nc.compile()
    return nc


def _get_nc():
    global _nc_cache
    if _nc_cache is None:
        _nc_cache = _build_program()
    return _nc_cache


def _prep_core_inputs(b, edge_index, x, edge_attr, weights):
    (We0, be0, Wn0, bn0, We1, be1, We2, be2) = weights
    A = edge_index[b].astype(F32)
    x0 = x[b].astype(F32)

    A2 = A.reshape(2, 128, 256)                       # [r, q, j]
    am1 = (A2 - 1.0).reshape(2, NPAIR * 256).astype(BF16)

    x0t = x0.T.astype(F32)                            # [64, 256]
    x0t2 = np.tile(x0t, (1, 2)).astype(BF16)

    Axi0 = (x0 @ We0[0:64]).T + be0[:, None]          # [64, 256]
    bias0 = np.concatenate([Axi0[:, 0:128], Axi0[:, 128:256]], 0).astype(F32)

    deg = np.clip(A.sum(1), 1.0, None)
    dinv = (1.0 / deg).astype(F32)
    dinvP = np.concatenate(
        [np.tile(dinv[None, 0:128], (64, 1)), np.tile(dinv[None, 128:256], (64, 1))], 0
    ).astype(F32)

    def bdiag(We):
        Wee = We[128:192]
        out = np.zeros((128, 128), F32)
        out[0:64, 0:64] = Wee
        out[64:128, 64:128] = Wee
        return out.astype(BF16)

    big2 = np.zeros((2, 128), F32)
    big2[0, 0:64] = BIGV
    big2[1, 64:128] = BIGV

    def w23(We, masked):
        wxj2 = np.tile(We[64:128], (1, 2))
        rows = big2 if masked else np.zeros((2, 128), F32)
        return np.concatenate([wxj2, rows], 0).astype(BF16)

    def w23rep(We, masked):
        base = w23(We, masked).astype(F32)          # [66, 128]
        rep = np.tile(base, (1, 64))                # [66, 8192]
        out = np.zeros((68, 8192), F32)
        out[0:66] = rep
        return out.astype(BF16)

    ind2 = np.zeros((2, QC * 256), F32)
    ind2[0].reshape(8, 512)[:, 0:256] = 1.0
    ind2[1].reshape(8, 512)[:, 256:512] = 1.0

    return {
        # host pre-arrangement into feature-major pair-tiles:
        # e0[r*64+f, q*256+j] = edge_attr[q+128r, j, f]
        "e0": np.ascontiguousarray(
            edge_attr[b].astype(F32)
            .reshape(2, 128, 256, FE)
            .transpose(0, 3, 1, 2)
            .reshape(128, 128 * 256)
        ),
        "am1": am1,
        "x0t2": x0t2,
        "bias0": bias0,
        "dinvP": dinvP,
        "bd0": bdiag(We0),
        "bd1": bdiag(We1),
        "bd2": bdiag(We2),
        "w23_0": w23(We0, True),
        "w23rep_1": w23rep(We1, False),
        "w23rep_2": w23rep(We2, True),
        "ind2": ind2.astype(BF16),
        "wxibe1": np.concatenate([We1[0:64], be1[None, :]], 0).astype(BF16),
        "wxibe2": np.concatenate([We2[0:64], be2[None, :]], 0).astype(BF16),
        "wn0x": Wn0[0:64].astype(BF16),
        "wn0a": Wn0[64:128].astype(BF16),
        "wn0a2": np.concatenate([np.zeros((64, 64), F32), Wn0[64:128]], 0).astype(BF16),
        "bn0c": bn0[:, None].astype(F32),
    }


def run_traced(edge_index, x, edge_attr,
               We0, be0, Wn0, bn0,
               We1, be1, Wn1, bn1,
               We2, be2, Wn2, bn2,
               W1, b1, W2, b2, W3, b3, **kw):
    """Correctness + profiling run; returns (out, BassKernelResults)."""
    nc = _get_nc()
    weights = tuple(
        np.asarray(w, F32)
        for w in (We0, be0, Wn0, bn0, We1, be1, We2, be2)
    )
    in_maps = [
        _prep_core_inputs(b, np.asarray(edge_index), np.asarray(x),
                          np.asarray(edge_attr), weights)
        for b in range(B)
    ]
    res = run_bass_kernel_spmd(
        nc, in_maps, core_ids=list(range(B)), trace=True
    )
    return res


def kernel(edge_index, x, edge_attr,
           We0, be0, Wn0, bn0,
           We1, be1, Wn1, bn1,
           We2, be2, Wn2, bn2,
           W1, b1, W2, b2, W3, b3, **kw):
    nc = _get_nc()
    weights = tuple(
        np.asarray(w, F32)
        for w in (We0, be0, Wn0, bn0, We1, be1, We2, be2)
    )
    in_maps = [
        _prep_core_inputs(b, np.asarray(edge_index), np.asarray(x),
                          np.asarray(edge_attr), weights)
        for b in range(B)
    ]
    res = run_bass_kernel_spmd(nc, in_maps, core_ids=list(range(B)))
    out = np.zeros((B,), F32)
    for b in range(B):
        vc = res.results[b]["vcols"].astype(F32)
        v128 = vc.sum(1)
        v = (v128[:64] + v128[64:]) / float(N * N)
        h = _relu(v @ np.asarray(W1, F32) + np.asarray(b1, F32))
        h = _relu(h @ np.asarray(W2, F32) + np.asarray(b2, F32))
        out[b] = (h @ np.asarray(W3, F32) + np.asarray(b3, F32))[0]
    return out
